# revision 1
# baseline (speedup 1.0000x reference)
"""7x7 valid conv2d on [8192, 8192] fp32, distributed over 8 NeuronCores.

Strategy: row-shard the image across 8 cores (1024 output rows each; host-side
overlapping slices provide the 6-row halo). On each core the conv runs as
patch-packed matmuls in bf16: the host packs x into 16x8 pixel patches laid
across the 128 SBUF partitions (partition p = dr*8+dc, free axis = patch
index). Each 16x8 output patch draws on the 2x2 neighborhood of input patches,
so 4 accumulating matmuls with per-alignment stationary matrices B_q[128,128]
produce 128 output pixels per streamed column — 4 PE cycles per 128 outputs vs
the banded-Toeplitz formulation's 7 per 122. Bias is folded into the
PSUM->SBUF copy; the output leaves the device patch-packed bf16 and the host
unpacks/upcasts.
"""

import numpy as np

KH = KW = 7
H = W = 8192
OH = H - KH + 1  # 8186
OW = W - KW + 1
NCORES = 8
P = 128

PR, PC = 16, 8          # patch rows x cols = 128 pixels
BAND = 1024             # output rows per core (last core overlaps)
NA = BAND // PR         # 64 output row-patches per core
NAI = NA + 1            # 65 input row-bands (one halo band)
NB = W // PC            # 1024 col-patches of output (host trims to OW)
NBI = NB + 1            # 1025 input col-patches (one halo patch)
N_TILE = 512            # output patches per PSUM tile (1 bank)

REPS = 1                # body repetitions (for slope timing only)
X_BUFS = 7              # input band buffers (a, a+1 live + 4-deep prefetch)
PREFETCH = 4            # bands loaded ahead of the consuming iteration
PS_BUFS = 8
O_BUFS = 4

ALIGNS = [(0, 0), (0, 1), (1, 0), (1, 1)]


def _build_program(bias_val):
    import concourse.bacc as bacc
    import concourse.mybir as mybir
    import concourse.tile as tile

    bf16 = mybir.dt.bfloat16
    f32 = mybir.dt.float32

    nc = bacc.Bacc(
        "TRN2",
        target_bir_lowering=False,
        debug=False,
        enable_asserts=False,
        num_devices=NCORES,
    )

    x_dram = nc.dram_tensor("xp", [P, NAI * NBI], bf16, kind="ExternalInput")
    w_dram = nc.dram_tensor("wq", [P, 4, P], bf16, kind="ExternalInput")
    out_dram = nc.dram_tensor("out", [P, NA * NB], bf16, kind="ExternalOutput")

    with tile.TileContext(nc) as tc:
        with (
            tc.tile_pool(name="const", bufs=1) as cpool,
            tc.tile_pool(name="xp", bufs=X_BUFS) as xpool,
            tc.tile_pool(name="op", bufs=O_BUFS) as opool,
            tc.tile_pool(name="pp", bufs=PS_BUFS, space="PSUM") as pspool,
        ):
            w_sb = cpool.tile([P, 4, P], bf16)
            nc.scalar.dma_start(w_sb[:], w_dram.ap()[:])
            bias_sb = cpool.tile([P, 1], f32)
            nc.vector.memset(bias_sb[:], bias_val)

            for _rep in range(REPS):
                def load_band(a, eng=None):
                    t = xpool.tile([P, NBI], bf16, tag="x", name="x_sb")
                    (eng or nc.sync).dma_start(
                        t[:], x_dram.ap()[:, a * NBI : (a + 1) * NBI]
                    )
                    return t

                def load_band_split(a, eng_lo, eng_hi):
                    # halve the first loads across two DGE queues so the
                    # first matmul's inputs land sooner
                    t = xpool.tile([P, NBI], bf16, tag="x", name="x_sb")
                    half = NBI // 2
                    c0 = a * NBI
                    eng_lo.dma_start(t[:, :half], x_dram.ap()[:, c0 : c0 + half])
                    eng_hi.dma_start(
                        t[:, half:], x_dram.ap()[:, c0 + half : c0 + NBI]
                    )
                    return t

                bands = {
                    0: load_band_split(0, nc.sync, nc.gpsimd),
                    1: load_band_split(1, nc.sync, nc.gpsimd),
                    2: load_band(2, nc.sync),
                    3: load_band(3, nc.gpsimd),
                    4: load_band(4, nc.scalar),
                }
                for a in range(NA):
                    if a + 1 + PREFETCH < NAI:
                        bands[a + 1 + PREFETCH] = load_band(a + 1 + PREFETCH)
                    o_sb = opool.tile([P, NB], bf16, tag="o", name="o_sb")
                    for ti, b0 in enumerate(range(0, NB, N_TILE)):
                        ps = pspool.tile([P, N_TILE], f32, tag="ps", name="ps")
                        for q, (qa, qb) in enumerate(ALIGNS):
                            nc.tensor.matmul(
                                ps[:],
                                w_sb[:, q, :],
                                bands[a + qa][:, b0 + qb : b0 + qb + N_TILE],
                                start=(q == 0),
                                stop=(q == 3),
                            )
                        if (a + ti) % 2 == 0:
                            nc.scalar.activation(
                                o_sb[:, b0 : b0 + N_TILE], ps[:],
                                mybir.ActivationFunctionType.Identity,
                                bias=bias_sb[:],
                            )
                        else:
                            nc.vector.tensor_scalar_add(
                                o_sb[:, b0 : b0 + N_TILE], ps[:], bias_val
                            )
                        if a == NA - 1:
                            # drain the last band per-tile so the epilogue
                            # fence isn't gated on one late large DMA
                            nc.gpsimd.dma_start(
                                out_dram.ap()[:, a * NB + b0 : a * NB + b0 + N_TILE],
                                o_sb[:, b0 : b0 + N_TILE],
                            )
                    bands.pop(a, None)
                    if a < NA - 1:
                        nc.gpsimd.dma_start(
                            out_dram.ap()[:, a * NB : (a + 1) * NB], o_sb[:]
                        )

    nc.compile()
    return nc


def _bf16():
    import ml_dtypes

    return ml_dtypes.bfloat16


def _make_wq(weight):
    """Stationary matrices B_q[p, m]: p = dr*8+dc input pixel of patch
    (a+qa, b+qb), m = or*8+oc output pixel of patch (a, b). Returned in
    DRAM layout [p, q, m] bf16."""
    wq = np.zeros((4, P, P), np.float32)
    dr, dc = np.arange(PR)[:, None], np.arange(PC)[None, :]
    orr, occ = np.arange(PR)[:, None], np.arange(PC)[None, :]
    for q, (qa, qb) in enumerate(ALIGNS):
        # dh = PR*qa + dr - or, dw = PC*qb + dc - oc; valid in [0, 6]
        dh = PR * qa + dr.reshape(-1, 1, 1, 1) - orr.reshape(1, 1, -1, 1)
        dw = PC * qb + dc.reshape(1, -1, 1, 1) - occ.reshape(1, 1, 1, -1)
        valid = (dh >= 0) & (dh < KH) & (dw >= 0) & (dw < KW)
        vals = np.where(valid, weight[np.clip(dh, 0, KH - 1), np.clip(dw, 0, KW - 1)], 0.0)
        wq[q] = vals.reshape(P, P)
    return np.ascontiguousarray(wq.transpose(1, 0, 2)).astype(_bf16())


def _pack_x(x16, s):
    """Pack band starting at row s into [128, NAI*NBI] bf16.
    x16: full [H, W] bf16 array."""
    bf16 = _bf16()
    rows = PR * NAI
    xb = np.zeros((rows, PC * NBI), bf16)
    avail = min(rows, H - s)
    xb[:avail, :W] = x16[s : s + avail]
    v = xb.reshape(NAI, PR, NBI, PC)
    return np.ascontiguousarray(v.transpose(1, 3, 0, 2)).reshape(P, NAI * NBI)


def unpack_out(o_packed):
    """[128, NA*NB] bf16 -> [BAND, OW] fp32 band."""
    v = np.asarray(o_packed).reshape(PR, PC, NA, NB)
    band = v.transpose(2, 0, 3, 1).reshape(BAND, NB * PC)
    return band[:, :OW].astype(np.float32)


def make_in_maps(x, weight, starts):
    x16 = np.asarray(x, np.float32).astype(_bf16())
    wq = _make_wq(np.asarray(weight, np.float32))
    return [{"xp": _pack_x(x16, s), "wq": wq} for s in starts]


class Runner:
    """Compiles the per-core program once and exposes repeatable execution
    on all cores via PJRT (the axon path of run_bass_kernel_spmd, inlined so
    inputs can stay device-resident and calls can be timed)."""

    def __init__(self, bias_val):
        self._setup(_build_program(bias_val), NCORES)

    @classmethod
    def from_nc(cls, nc, n_cores=NCORES):
        r = cls.__new__(cls)
        r._setup(nc, n_cores)
        return r

    def _setup(self, nc, n_cores):
        import jax
        import concourse.mybir as mybir
        from concourse import bass2jax
        from jax.sharding import Mesh, PartitionSpec
        from jax.experimental.shard_map import shard_map

        self.n_cores = n_cores
        self.nc = nc
        bass2jax.install_neuronx_cc_hook()

        partition_name = (
            nc.partition_id_tensor.name if nc.partition_id_tensor else None
        )
        in_names, out_names, out_avals = [], [], []
        for alloc in nc.m.functions[0].allocations:
            if not isinstance(alloc, mybir.MemoryLocationSet):
                continue
            name = alloc.memorylocations[0].name
            if alloc.kind == "ExternalInput":
                if name != partition_name:
                    in_names.append(name)
            elif alloc.kind == "ExternalOutput":
                out_names.append(name)
                out_avals.append(
                    jax.core.ShapedArray(
                        tuple(alloc.tensor_shape), mybir.dt.np(alloc.dtype)
                    )
                )
        self.in_names, self.out_names, self.out_avals = in_names, out_names, out_avals
        n_params = len(in_names)
        donate = tuple(range(n_params, n_params + len(out_names)))

        def _body(*args):
            operands = list(args)
            if nc.partition_id_tensor is not None:
                operands.append(bass2jax.partition_id_tensor())
            outs = bass2jax._bass_exec_p.bind(
                *operands,
                out_avals=tuple(out_avals),
                in_names=tuple(in_names + out_names)
                + ((nc.partition_id_tensor.name,) if nc.partition_id_tensor else ()),
                out_names=tuple(out_names),
                lowering_input_output_aliases=(),
                sim_require_finite=True,
                sim_require_nnan=True,
                nc=nc,
            )
            return tuple(outs)

        devices = jax.devices()[:n_cores]
        self.mesh = Mesh(np.asarray(devices), ("core",))
        self.pspec = PartitionSpec("core")
        in_specs = (self.pspec,) * (n_params + len(out_names))
        out_specs = (self.pspec,) * len(out_names)
        self.fn = jax.jit(
            shard_map(
                _body,
                mesh=self.mesh,
                in_specs=in_specs,
                out_specs=out_specs,
                check_rep=False,
            ),
            donate_argnums=donate,
            keep_unused=True,
        )

    def put_inputs(self, in_maps):
        """device_put per-core input dicts; returns list of jax arrays."""
        import jax
        from jax.sharding import NamedSharding

        sharding = NamedSharding(self.mesh, self.pspec)
        arrs = []
        for name in self.in_names:
            cat = np.concatenate([np.asarray(m[name]) for m in in_maps], axis=0)
            arrs.append(jax.device_put(cat, sharding))
        return arrs

    def zero_outs(self):
        import jax
        from jax.sharding import NamedSharding

        sharding = NamedSharding(self.mesh, self.pspec)
        return tuple(
            jax.device_put(
                np.zeros((self.n_cores * a.shape[0], *a.shape[1:]), a.dtype), sharding
            )
            for a in self.out_avals
        )

    def run(self, in_arrs, out_bufs):
        """One execution; returns new device output arrays (donates out_bufs)."""
        return self.fn(*in_arrs, *out_bufs)

    def gather(self, outs):
        """Device outputs -> list of per-core dicts of np arrays."""
        res = []
        for c in range(self.n_cores):
            d = {}
            for i, name in enumerate(self.out_names):
                a = self.out_avals[i]
                d[name] = np.asarray(outs[i]).reshape(self.n_cores, *a.shape)[c]
            res.append(d)
        return res


def kernel(x, weight, bias):
    from concourse import bass_utils

    x = np.asarray(x, dtype=np.float32)
    weight = np.asarray(weight, dtype=np.float32)
    bias = np.asarray(bias, dtype=np.float32)

    starts = [min(i * BAND, OH - BAND) for i in range(NCORES)]
    nc = _build_program(float(bias[0]))
    res = bass_utils.run_bass_kernel_spmd(
        nc, make_in_maps(x, weight, starts), core_ids=list(range(NCORES))
    )

    out = np.empty((OH, OW), np.float32)
    for s, r in zip(starts, res.results):
        out[s : s + BAND] = unpack_out(r["out"])
    return out



# revision 29
# speedup vs baseline: 1.3777x; 1.3777x over previous
"""7x7 valid conv2d on [8192, 8192] fp32, distributed over 8 NeuronCores.

Strategy: row-shard the image across 8 cores (1024 output rows each; host-side
overlapping slices provide the 6-row halo). On each core the conv runs as
patch-packed matmuls in bf16: the host packs x into 16x8 pixel patches laid
across the 128 SBUF partitions (partition p = dr*8+dc, free axis = patch
index). Each 16x8 output patch draws on the 2x2 neighborhood of input patches,
so 4 accumulating matmuls with per-alignment stationary matrices B_q[128,128]
produce 128 output pixels per streamed column — 4 PE cycles per 128 outputs vs
the banded-Toeplitz formulation's 7 per 122. Bias is folded into the
PSUM->SBUF copy; the output leaves the device patch-packed bf16 and the host
unpacks/upcasts.
"""

import numpy as np

KH = KW = 7
H = W = 8192
OH = H - KH + 1  # 8186
OW = W - KW + 1
NCORES = 8
P = 128

PR, PC = 16, 8          # patch rows x cols = 128 pixels
BAND = 1024             # output rows per core (last core overlaps)
NA = BAND // PR         # 64 output row-patches per core
NAI = NA + 1            # 65 input row-bands (one halo band)
NB = W // PC            # 1024 col-patches of output (host trims to OW)
NBI = NB + 1            # 1025 input col-patches (one halo patch)
N_TILE = 512            # output patches per PSUM tile (1 bank)

REPS = 1                # body repetitions (for slope timing only)
WARMUP = 36             # PE warmup matmuls during the initial DMA window
X_BUFS = 7              # input band buffers (a, a+1 live + 4-deep prefetch)
PREFETCH = 4            # bands loaded ahead of the consuming iteration
PS_BUFS = 8
O_BUFS = 4

ALIGNS = [(0, 0), (0, 1), (1, 0), (1, 1)]


def _build_program(bias_val):
    import concourse.bacc as bacc
    import concourse.mybir as mybir
    import concourse.tile as tile

    bf16 = mybir.dt.bfloat16
    f32 = mybir.dt.float32

    nc = bacc.Bacc(
        "TRN2",
        target_bir_lowering=False,
        debug=False,
        enable_asserts=False,
        num_devices=NCORES,
    )

    x_dram = nc.dram_tensor("xp", [P, NAI * NBI], bf16, kind="ExternalInput")
    w_dram = nc.dram_tensor("wq", [P, 4, P], bf16, kind="ExternalInput")
    out_dram = nc.dram_tensor("out", [P, NA * NB], bf16, kind="ExternalOutput")

    with tile.TileContext(nc) as tc:
        with (
            tc.tile_pool(name="const", bufs=1) as cpool,
            tc.tile_pool(name="xp", bufs=X_BUFS) as xpool,
            tc.tile_pool(name="op", bufs=O_BUFS) as opool,
            tc.tile_pool(name="pp", bufs=PS_BUFS, space="PSUM") as pspool,
        ):
            # PE warmup: dummy matmuls keep the tensor engine continuously
            # busy from ~t=0, so the p-state ramp runs during the initial
            # input-DMA window instead of eating into the first real tiles.
            # Zeroed via the Pool engine (fastest queue bring-up, ~100ns vs
            # DVE's ~800ns) and issued before anything else on that queue;
            # the products land in a PSUM bank that is reset (start=True)
            # before any real accumulation.
            wz = cpool.tile([P, P], bf16)
            nc.gpsimd.memset(wz[:], 0.0)
            wps = pspool.tile([P, N_TILE], f32, tag="ps", name="ps")
            for _ in range(WARMUP):
                nc.tensor.matmul(wps[:, :P], wz[:], wz[:], start=True, stop=True)

            w_sb = cpool.tile([P, 4, P], bf16)
            # per-alignment chunks so the q=0 stationary (the first real
            # matmul's dep) lands first; q2/q3 ride gpsimd's SWDGE which runs
            # in parallel with the shared HWDGE descriptor pipeline
            for q, eng in enumerate((nc.sync, nc.scalar, nc.gpsimd, nc.gpsimd)):
                eng.dma_start(w_sb[:, q, :], w_dram.ap()[:, q, :])
            bias_sb = cpool.tile([P, 1], f32)
            nc.vector.memset(bias_sb[:], bias_val)

            # tile spans per band: the first band starts with small tiles so
            # the first matmul only waits on one quarter-band DMA per input
            # band; the last band ends with small tiles so the final
            # eviction+drain tail is short.
            spans_full = [(b0, N_TILE) for b0 in range(0, NB, N_TILE)]
            spans_last = spans_full[:-1] + [
                (NB - N_TILE, 256), (NB - 256, 128), (NB - 128, 128)
            ]

            for _rep in range(REPS):
                def load_band(a, eng=None):
                    t = xpool.tile([P, NBI], bf16, tag="x", name="x_sb")
                    (eng or nc.sync).dma_start(
                        t[:], x_dram.ap()[:, a * NBI : (a + 1) * NBI]
                    )
                    return t

                # whole-band loads, one descriptor each; bands 0 and 1 lead
                # the two HWDGE queues so the first tile's deps land first
                bands = {
                    0: load_band(0, nc.sync),
                    1: load_band(1, nc.scalar),
                    2: load_band(2, nc.sync),
                    3: load_band(3, nc.scalar),
                    4: load_band(4, nc.gpsimd),
                }
                for a in range(NA):
                    if a + 1 + PREFETCH < NAI:
                        bands[a + 1 + PREFETCH] = load_band(
                            a + 1 + PREFETCH,
                            nc.sync if a % 2 == 0 else nc.scalar,
                        )
                    o_sb = opool.tile([P, NB], bf16, tag="o", name="o_sb")
                    last = a == NA - 1
                    spans = spans_last if last else spans_full
                    for ti, (b0, tw) in enumerate(spans):
                        ps = pspool.tile([P, N_TILE], f32, tag="ps", name="ps")
                        for q, (qa, qb) in enumerate(ALIGNS):
                            nc.tensor.matmul(
                                ps[:, :tw],
                                w_sb[:, q, :],
                                bands[a + qa][:, b0 + qb : b0 + qb + tw],
                                start=(q == 0),
                                stop=(q == 3),
                            )
                        if (a + ti) % 2 == 0:
                            nc.scalar.activation(
                                o_sb[:, b0 : b0 + tw], ps[:, :tw],
                                mybir.ActivationFunctionType.Identity,
                                bias=bias_sb[:],
                            )
                        else:
                            nc.vector.tensor_scalar_add(
                                o_sb[:, b0 : b0 + tw], ps[:, :tw], bias_val
                            )
                    bands.pop(a, None)
                    if last:
                        # single descriptor on the HWDGE ring right after the
                        # final (small) eviction: serial ~630ns descriptor
                        # processing makes one large drain faster than four
                        # small ones at the epilogue
                        nc.sync.dma_start(
                            out_dram.ap()[:, a * NB : (a + 1) * NB], o_sb[:]
                        )
                    else:
                        nc.gpsimd.dma_start(
                            out_dram.ap()[:, a * NB : (a + 1) * NB], o_sb[:]
                        )

    nc.compile()
    return nc


def _bf16():
    import ml_dtypes

    return ml_dtypes.bfloat16


def _make_wq(weight):
    """Stationary matrices B_q[p, m]: p = dr*8+dc input pixel of patch
    (a+qa, b+qb), m = or*8+oc output pixel of patch (a, b). Returned in
    DRAM layout [p, q, m] bf16."""
    wq = np.zeros((4, P, P), np.float32)
    dr, dc = np.arange(PR)[:, None], np.arange(PC)[None, :]
    orr, occ = np.arange(PR)[:, None], np.arange(PC)[None, :]
    for q, (qa, qb) in enumerate(ALIGNS):
        # dh = PR*qa + dr - or, dw = PC*qb + dc - oc; valid in [0, 6]
        dh = PR * qa + dr.reshape(-1, 1, 1, 1) - orr.reshape(1, 1, -1, 1)
        dw = PC * qb + dc.reshape(1, -1, 1, 1) - occ.reshape(1, 1, 1, -1)
        valid = (dh >= 0) & (dh < KH) & (dw >= 0) & (dw < KW)
        vals = np.where(valid, weight[np.clip(dh, 0, KH - 1), np.clip(dw, 0, KW - 1)], 0.0)
        wq[q] = vals.reshape(P, P)
    return np.ascontiguousarray(wq.transpose(1, 0, 2)).astype(_bf16())


def _pack_x(x16, s):
    """Pack band starting at row s into [128, NAI*NBI] bf16.
    x16: full [H, W] bf16 array."""
    bf16 = _bf16()
    rows = PR * NAI
    xb = np.zeros((rows, PC * NBI), bf16)
    avail = min(rows, H - s)
    xb[:avail, :W] = x16[s : s + avail]
    v = xb.reshape(NAI, PR, NBI, PC)
    return np.ascontiguousarray(v.transpose(1, 3, 0, 2)).reshape(P, NAI * NBI)


def unpack_out(o_packed):
    """[128, NA*NB] bf16 -> [BAND, OW] fp32 band."""
    v = np.asarray(o_packed).reshape(PR, PC, NA, NB)
    band = v.transpose(2, 0, 3, 1).reshape(BAND, NB * PC)
    return band[:, :OW].astype(np.float32)


def make_in_maps(x, weight, starts):
    x16 = np.asarray(x, np.float32).astype(_bf16())
    wq = _make_wq(np.asarray(weight, np.float32))
    return [{"xp": _pack_x(x16, s), "wq": wq} for s in starts]


class Runner:
    """Compiles the per-core program once and exposes repeatable execution
    on all cores via PJRT (the axon path of run_bass_kernel_spmd, inlined so
    inputs can stay device-resident and calls can be timed)."""

    def __init__(self, bias_val):
        self._setup(_build_program(bias_val), NCORES)

    @classmethod
    def from_nc(cls, nc, n_cores=NCORES):
        r = cls.__new__(cls)
        r._setup(nc, n_cores)
        return r

    def _setup(self, nc, n_cores):
        import jax
        import concourse.mybir as mybir
        from concourse import bass2jax
        from jax.sharding import Mesh, PartitionSpec
        from jax.experimental.shard_map import shard_map

        self.n_cores = n_cores
        self.nc = nc
        bass2jax.install_neuronx_cc_hook()

        partition_name = (
            nc.partition_id_tensor.name if nc.partition_id_tensor else None
        )
        in_names, out_names, out_avals = [], [], []
        for alloc in nc.m.functions[0].allocations:
            if not isinstance(alloc, mybir.MemoryLocationSet):
                continue
            name = alloc.memorylocations[0].name
            if alloc.kind == "ExternalInput":
                if name != partition_name:
                    in_names.append(name)
            elif alloc.kind == "ExternalOutput":
                out_names.append(name)
                out_avals.append(
                    jax.core.ShapedArray(
                        tuple(alloc.tensor_shape), mybir.dt.np(alloc.dtype)
                    )
                )
        self.in_names, self.out_names, self.out_avals = in_names, out_names, out_avals
        n_params = len(in_names)
        donate = tuple(range(n_params, n_params + len(out_names)))

        def _body(*args):
            operands = list(args)
            if nc.partition_id_tensor is not None:
                operands.append(bass2jax.partition_id_tensor())
            outs = bass2jax._bass_exec_p.bind(
                *operands,
                out_avals=tuple(out_avals),
                in_names=tuple(in_names + out_names)
                + ((nc.partition_id_tensor.name,) if nc.partition_id_tensor else ()),
                out_names=tuple(out_names),
                lowering_input_output_aliases=(),
                sim_require_finite=True,
                sim_require_nnan=True,
                nc=nc,
            )
            return tuple(outs)

        devices = jax.devices()[:n_cores]
        self.mesh = Mesh(np.asarray(devices), ("core",))
        self.pspec = PartitionSpec("core")
        in_specs = (self.pspec,) * (n_params + len(out_names))
        out_specs = (self.pspec,) * len(out_names)
        self.fn = jax.jit(
            shard_map(
                _body,
                mesh=self.mesh,
                in_specs=in_specs,
                out_specs=out_specs,
                check_rep=False,
            ),
            donate_argnums=donate,
            keep_unused=True,
        )

    def put_inputs(self, in_maps):
        """device_put per-core input dicts; returns list of jax arrays."""
        import jax
        from jax.sharding import NamedSharding

        sharding = NamedSharding(self.mesh, self.pspec)
        arrs = []
        for name in self.in_names:
            cat = np.concatenate([np.asarray(m[name]) for m in in_maps], axis=0)
            arrs.append(jax.device_put(cat, sharding))
        return arrs

    def zero_outs(self):
        import jax
        from jax.sharding import NamedSharding

        sharding = NamedSharding(self.mesh, self.pspec)
        return tuple(
            jax.device_put(
                np.zeros((self.n_cores * a.shape[0], *a.shape[1:]), a.dtype), sharding
            )
            for a in self.out_avals
        )

    def run(self, in_arrs, out_bufs):
        """One execution; returns new device output arrays (donates out_bufs)."""
        return self.fn(*in_arrs, *out_bufs)

    def gather(self, outs):
        """Device outputs -> list of per-core dicts of np arrays."""
        res = []
        for c in range(self.n_cores):
            d = {}
            for i, name in enumerate(self.out_names):
                a = self.out_avals[i]
                d[name] = np.asarray(outs[i]).reshape(self.n_cores, *a.shape)[c]
            res.append(d)
        return res


def kernel(x, weight, bias):
    from concourse import bass_utils

    x = np.asarray(x, dtype=np.float32)
    weight = np.asarray(weight, dtype=np.float32)
    bias = np.asarray(bias, dtype=np.float32)

    starts = [min(i * BAND, OH - BAND) for i in range(NCORES)]
    nc = _build_program(float(bias[0]))
    res = bass_utils.run_bass_kernel_spmd(
        nc, make_in_maps(x, weight, starts), core_ids=list(range(NCORES))
    )

    out = np.empty((OH, OW), np.float32)
    for s, r in zip(starts, res.results):
        out[s : s + BAND] = unpack_out(r["out"])
    return out



# revision 38
# speedup vs baseline: 1.4382x; 1.0439x over previous
"""7x7 valid conv2d on [8192, 8192] fp32, distributed over 8 NeuronCores.

Strategy: row-shard the image across 8 cores (1024 output rows each; host-side
overlapping slices provide the 6-row halo). On each core the conv runs as
patch-packed matmuls in bf16: the host packs x into 16x8 pixel patches laid
across the 128 SBUF partitions (partition p = dr*8+dc, free axis = patch
index). Each 16x8 output patch draws on the 2x2 neighborhood of input patches,
so 4 accumulating matmuls with per-alignment stationary matrices B_q[128,128]
produce 128 output pixels per streamed column — 4 PE cycles per 128 outputs vs
the banded-Toeplitz formulation's 7 per 122. Bias is folded into the
PSUM->SBUF copy; the output leaves the device patch-packed bf16 and the host
unpacks/upcasts.
"""

import numpy as np

KH = KW = 7
H = W = 8192
OH = H - KH + 1  # 8186
OW = W - KW + 1
NCORES = 8
P = 128

PR, PC = 16, 8          # patch rows x cols = 128 pixels
BAND = 1024             # output rows per core (last core overlaps)
NA = BAND // PR         # 64 output row-patches per core
NAI = NA + 1            # 65 input row-bands (one halo band)
NB = W // PC            # 1024 col-patches of output (host trims to OW)
NBI = NB + 1            # 1025 input col-patches (one halo patch)
N_TILE = 512            # output patches per PSUM tile (1 bank)

REPS = 1                # body repetitions (for slope timing only)
WARMUP = 44             # PE warmup matmuls during the initial DMA window
NPAIRS = NAI // 2 + 1   # 33 band-pair tiles (the last holds one band)
X_BUFS = 5              # pair buffers (2 live + 2-3 prefetched)
PS_BUFS = 8
O_BUFS = 3              # output pair buffers

ALIGNS = [(0, 0), (0, 1), (1, 0), (1, 1)]


def _build_program(bias_val):
    import concourse.bacc as bacc
    import concourse.mybir as mybir
    import concourse.tile as tile

    bf16 = mybir.dt.bfloat16
    f32 = mybir.dt.float32

    nc = bacc.Bacc(
        "TRN2",
        target_bir_lowering=False,
        debug=False,
        enable_asserts=False,
        num_devices=NCORES,
    )

    x_dram = nc.dram_tensor("xp", [P, NAI * NBI], bf16, kind="ExternalInput")
    w_dram = nc.dram_tensor("wq", [P, 4, P], bf16, kind="ExternalInput")
    out_dram = nc.dram_tensor("out", [P, NA * NB], bf16, kind="ExternalOutput")

    with tile.TileContext(nc) as tc:
        with (
            tc.tile_pool(name="const", bufs=1) as cpool,
            tc.tile_pool(name="xp", bufs=X_BUFS) as xpool,
            tc.tile_pool(name="op", bufs=O_BUFS) as opool,
            tc.tile_pool(name="pp", bufs=PS_BUFS, space="PSUM") as pspool,
        ):
            # PE warmup: dummy matmuls keep the tensor engine continuously
            # busy from ~t=0, so the p-state ramp runs during the initial
            # input-DMA window instead of eating into the first real tiles.
            # Zeroed via the Pool engine (fastest queue bring-up, ~100ns vs
            # DVE's ~800ns) and issued before anything else on that queue;
            # the products land in a PSUM bank that is reset (start=True)
            # before any real accumulation.
            wz = cpool.tile([P, P], bf16)
            nc.gpsimd.memset(wz[:], 0.0)
            wps = pspool.tile([P, N_TILE], f32, tag="ps", name="ps")
            for _ in range(WARMUP):
                nc.tensor.matmul(wps[:, :P], wz[:], wz[:], start=True, stop=True)

            w_sb = cpool.tile([P, 4, P], bf16)
            # q0/q1 ahead of the first input pair on the HWDGE ring; q2/q3
            # are only needed a few hundred ns later, after the first pair
            # lands, so they ride behind it
            for q, eng in enumerate((nc.sync, nc.scalar, nc.gpsimd, nc.gpsimd)):
                eng.dma_start(w_sb[:, q, :], w_dram.ap()[:, q, :])
            bias_sb = cpool.tile([P, 1], f32)
            nc.vector.memset(bias_sb[:], bias_val)

            # tile spans per band: the first band starts with small tiles so
            # the first matmul only waits on one quarter-band DMA per input
            # band; the last band ends with small tiles so the final
            # eviction+drain tail is short.
            spans_full = [(b0, N_TILE) for b0 in range(0, NB, N_TILE)]
            spans_last = spans_full[:-1] + [
                (NB - N_TILE, 256), (NB - 256, 128), (NB - 128, 128)
            ]

            for _rep in range(REPS):
                # band-PAIR loads: one descriptor per two bands halves the
                # DMA event count (descriptor processing is the serial
                # resource on both the HWDGE ring and gpsimd's SWDGE)
                def load_pair(k, eng=None):
                    t = xpool.tile([P, 2 * NBI], bf16, tag="x", name="x_sb")
                    w = 2 * NBI if k < NPAIRS - 1 else NBI
                    (eng or nc.sync).dma_start(
                        t[:, :w],
                        x_dram.ap()[:, 2 * k * NBI : 2 * k * NBI + w],
                    )
                    return t

                pairs = {
                    0: load_pair(0, nc.sync),
                    1: load_pair(1, nc.scalar),
                    2: load_pair(2, nc.gpsimd),
                }
                o_sb = None
                for a in range(NA):
                    if a % 2 == 0:
                        k = a // 2 + 3
                        if k < NPAIRS:
                            pairs[k] = load_pair(
                                k, nc.sync if (a // 2) % 2 == 0 else nc.scalar
                            )
                        o_sb = opool.tile([P, 2 * NB], bf16, tag="o", name="o_sb")
                    oo = (a % 2) * NB
                    last = a == NA - 1
                    spans = spans_last if last else spans_full
                    for ti, (b0, tw) in enumerate(spans):
                        ps = pspool.tile([P, N_TILE], f32, tag="ps", name="ps")
                        for q, (qa, qb) in enumerate(ALIGNS):
                            src = pairs[(a + qa) // 2]
                            off = ((a + qa) % 2) * NBI
                            nc.tensor.matmul(
                                ps[:, :tw],
                                w_sb[:, q, :],
                                src[:, off + b0 + qb : off + b0 + qb + tw],
                                start=(q == 0),
                                stop=(q == 3),
                            )
                        if (a + ti) % 2 == 0:
                            nc.scalar.activation(
                                o_sb[:, oo + b0 : oo + b0 + tw], ps[:, :tw],
                                mybir.ActivationFunctionType.Identity,
                                bias=bias_sb[:],
                            )
                        else:
                            nc.vector.tensor_scalar_add(
                                o_sb[:, oo + b0 : oo + b0 + tw], ps[:, :tw],
                                bias_val,
                            )
                    if a % 2 == 1:
                        pairs.pop(a // 2, None)
                    if a == NA - 2:
                        # drain the second-to-last band alone so the final
                        # band's drain (the tail) is as small as possible
                        nc.gpsimd.dma_start(
                            out_dram.ap()[:, a * NB : (a + 1) * NB],
                            o_sb[:, :NB],
                        )
                    elif last:
                        # single descriptor on the idle HWDGE ring right after
                        # the final (small) eviction
                        nc.sync.dma_start(
                            out_dram.ap()[:, a * NB : (a + 1) * NB],
                            o_sb[:, NB:],
                        )
                    elif a % 2 == 1:
                        nc.gpsimd.dma_start(
                            out_dram.ap()[:, (a - 1) * NB : (a + 1) * NB],
                            o_sb[:],
                        )

    nc.compile()
    return nc


def _bf16():
    import ml_dtypes

    return ml_dtypes.bfloat16


def _make_wq(weight):
    """Stationary matrices B_q[p, m]: p = dr*8+dc input pixel of patch
    (a+qa, b+qb), m = or*8+oc output pixel of patch (a, b). Returned in
    DRAM layout [p, q, m] bf16."""
    wq = np.zeros((4, P, P), np.float32)
    dr, dc = np.arange(PR)[:, None], np.arange(PC)[None, :]
    orr, occ = np.arange(PR)[:, None], np.arange(PC)[None, :]
    for q, (qa, qb) in enumerate(ALIGNS):
        # dh = PR*qa + dr - or, dw = PC*qb + dc - oc; valid in [0, 6]
        dh = PR * qa + dr.reshape(-1, 1, 1, 1) - orr.reshape(1, 1, -1, 1)
        dw = PC * qb + dc.reshape(1, -1, 1, 1) - occ.reshape(1, 1, 1, -1)
        valid = (dh >= 0) & (dh < KH) & (dw >= 0) & (dw < KW)
        vals = np.where(valid, weight[np.clip(dh, 0, KH - 1), np.clip(dw, 0, KW - 1)], 0.0)
        wq[q] = vals.reshape(P, P)
    return np.ascontiguousarray(wq.transpose(1, 0, 2)).astype(_bf16())


def _pack_x(x16, s):
    """Pack band starting at row s into [128, NAI*NBI] bf16.
    x16: full [H, W] bf16 array."""
    bf16 = _bf16()
    rows = PR * NAI
    xb = np.zeros((rows, PC * NBI), bf16)
    avail = min(rows, H - s)
    xb[:avail, :W] = x16[s : s + avail]
    v = xb.reshape(NAI, PR, NBI, PC)
    return np.ascontiguousarray(v.transpose(1, 3, 0, 2)).reshape(P, NAI * NBI)


def unpack_out(o_packed):
    """[128, NA*NB] bf16 -> [BAND, OW] fp32 band."""
    v = np.asarray(o_packed).reshape(PR, PC, NA, NB)
    band = v.transpose(2, 0, 3, 1).reshape(BAND, NB * PC)
    return band[:, :OW].astype(np.float32)


def make_in_maps(x, weight, starts):
    x16 = np.asarray(x, np.float32).astype(_bf16())
    wq = _make_wq(np.asarray(weight, np.float32))
    return [{"xp": _pack_x(x16, s), "wq": wq} for s in starts]


class Runner:
    """Compiles the per-core program once and exposes repeatable execution
    on all cores via PJRT (the axon path of run_bass_kernel_spmd, inlined so
    inputs can stay device-resident and calls can be timed)."""

    def __init__(self, bias_val):
        self._setup(_build_program(bias_val), NCORES)

    @classmethod
    def from_nc(cls, nc, n_cores=NCORES):
        r = cls.__new__(cls)
        r._setup(nc, n_cores)
        return r

    def _setup(self, nc, n_cores):
        import jax
        import concourse.mybir as mybir
        from concourse import bass2jax
        from jax.sharding import Mesh, PartitionSpec
        from jax.experimental.shard_map import shard_map

        self.n_cores = n_cores
        self.nc = nc
        bass2jax.install_neuronx_cc_hook()

        partition_name = (
            nc.partition_id_tensor.name if nc.partition_id_tensor else None
        )
        in_names, out_names, out_avals = [], [], []
        for alloc in nc.m.functions[0].allocations:
            if not isinstance(alloc, mybir.MemoryLocationSet):
                continue
            name = alloc.memorylocations[0].name
            if alloc.kind == "ExternalInput":
                if name != partition_name:
                    in_names.append(name)
            elif alloc.kind == "ExternalOutput":
                out_names.append(name)
                out_avals.append(
                    jax.core.ShapedArray(
                        tuple(alloc.tensor_shape), mybir.dt.np(alloc.dtype)
                    )
                )
        self.in_names, self.out_names, self.out_avals = in_names, out_names, out_avals
        n_params = len(in_names)
        donate = tuple(range(n_params, n_params + len(out_names)))

        def _body(*args):
            operands = list(args)
            if nc.partition_id_tensor is not None:
                operands.append(bass2jax.partition_id_tensor())
            outs = bass2jax._bass_exec_p.bind(
                *operands,
                out_avals=tuple(out_avals),
                in_names=tuple(in_names + out_names)
                + ((nc.partition_id_tensor.name,) if nc.partition_id_tensor else ()),
                out_names=tuple(out_names),
                lowering_input_output_aliases=(),
                sim_require_finite=True,
                sim_require_nnan=True,
                nc=nc,
            )
            return tuple(outs)

        devices = jax.devices()[:n_cores]
        self.mesh = Mesh(np.asarray(devices), ("core",))
        self.pspec = PartitionSpec("core")
        in_specs = (self.pspec,) * (n_params + len(out_names))
        out_specs = (self.pspec,) * len(out_names)
        self.fn = jax.jit(
            shard_map(
                _body,
                mesh=self.mesh,
                in_specs=in_specs,
                out_specs=out_specs,
                check_rep=False,
            ),
            donate_argnums=donate,
            keep_unused=True,
        )

    def put_inputs(self, in_maps):
        """device_put per-core input dicts; returns list of jax arrays."""
        import jax
        from jax.sharding import NamedSharding

        sharding = NamedSharding(self.mesh, self.pspec)
        arrs = []
        for name in self.in_names:
            cat = np.concatenate([np.asarray(m[name]) for m in in_maps], axis=0)
            arrs.append(jax.device_put(cat, sharding))
        return arrs

    def zero_outs(self):
        import jax
        from jax.sharding import NamedSharding

        sharding = NamedSharding(self.mesh, self.pspec)
        return tuple(
            jax.device_put(
                np.zeros((self.n_cores * a.shape[0], *a.shape[1:]), a.dtype), sharding
            )
            for a in self.out_avals
        )

    def run(self, in_arrs, out_bufs):
        """One execution; returns new device output arrays (donates out_bufs)."""
        return self.fn(*in_arrs, *out_bufs)

    def gather(self, outs):
        """Device outputs -> list of per-core dicts of np arrays."""
        res = []
        for c in range(self.n_cores):
            d = {}
            for i, name in enumerate(self.out_names):
                a = self.out_avals[i]
                d[name] = np.asarray(outs[i]).reshape(self.n_cores, *a.shape)[c]
            res.append(d)
        return res


def kernel(x, weight, bias):
    from concourse import bass_utils

    x = np.asarray(x, dtype=np.float32)
    weight = np.asarray(weight, dtype=np.float32)
    bias = np.asarray(bias, dtype=np.float32)

    starts = [min(i * BAND, OH - BAND) for i in range(NCORES)]
    nc = _build_program(float(bias[0]))
    res = bass_utils.run_bass_kernel_spmd(
        nc, make_in_maps(x, weight, starts), core_ids=list(range(NCORES))
    )

    out = np.empty((OH, OW), np.float32)
    for s, r in zip(starts, res.results):
        out[s : s + BAND] = unpack_out(r["out"])
    return out

